# revision 7
# baseline (speedup 1.0000x reference)
"""Trainium2 Bass kernel for nn_NodeBlock (GNN message passing).

Pipeline: segment_sum of edge features onto destination nodes, concat with
node features, 3-layer MLP, LayerNorm.

Sharding: nodes are range-sharded across the 8 cores (12800 nodes/core, 100
blocks of 128). On the host, edges are bucketed by destination-node block
(a shard of the edge list per core, padded per block to a uniform tile
count K), so each core streams only the edge rows it needs, contiguously.
On device, per 128-node block: the segment sum is a one-hot matmul
accumulated in PSUM (aggrT[f, j] = sum_e edge[e, f] * (col_local[e] == j)),
followed by the MLP in feature-major layout and a PE transpose + LayerNorm.
"""

import sys

sys.path.insert(0, "/opt/trn_rl_repo")

import numpy as np

N_CORES = 8
NUM_NODES = 100000
D = 128            # node/edge feature dim
P = 128            # partitions
BLK = 128          # nodes per block
BLOCKS_PER_CORE = 100
NODES_PER_CORE = BLK * BLOCKS_PER_CORE   # 12800
TOTAL_BLOCKS = N_CORES * BLOCKS_PER_CORE  # 800
EPS = 1e-5

_nc_cache = {}
last_run_info = {}


def _build_nc(K, loop_iters=None):
    import contextlib
    import concourse.bacc as bacc
    import concourse.tile as tile
    import concourse.mybir as mybir
    from concourse.masks import make_identity

    dt = mybir.dt
    f32 = dt.float32
    Alu = mybir.AluOpType
    Act = mybir.ActivationFunctionType
    KE = K * 128

    nc = bacc.Bacc("TRN2", target_bir_lowering=False, debug=False,
                   name="nodeblock")

    edges = nc.dram_tensor("edges", [BLOCKS_PER_CORE, P, KE], f32,
                           kind="ExternalInput")
    colf = nc.dram_tensor("colf", [P, BLOCKS_PER_CORE * K], f32,
                          kind="ExternalInput")
    natT = nc.dram_tensor("natT", [P, NODES_PER_CORE], f32,
                          kind="ExternalInput")
    iota = nc.dram_tensor("iota", [P, K, 128], f32, kind="ExternalInput")
    w0a = nc.dram_tensor("w0a", [128, 128], f32, kind="ExternalInput")
    w0b = nc.dram_tensor("w0b", [128, 128], f32, kind="ExternalInput")
    w1 = nc.dram_tensor("w1", [128, 128], f32, kind="ExternalInput")
    w2 = nc.dram_tensor("w2", [128, 128], f32, kind="ExternalInput")
    b0 = nc.dram_tensor("b0", [128, 1], f32, kind="ExternalInput")
    b1 = nc.dram_tensor("b1", [128, 1], f32, kind="ExternalInput")
    b2 = nc.dram_tensor("b2", [128, 1], f32, kind="ExternalInput")
    gam = nc.dram_tensor("gam", [128, 128], f32, kind="ExternalInput")
    bet = nc.dram_tensor("bet", [128, 128], f32, kind="ExternalInput")
    out = nc.dram_tensor("out", [BLOCKS_PER_CORE, P, D], f32,
                         kind="ExternalOutput")

    with tile.TileContext(nc) as tc:
        with (
            tc.tile_pool(name="const", bufs=1) as cpool,
            tc.tile_pool(name="edge", bufs=3) as epool,
            tc.tile_pool(name="oh", bufs=2) as ohpool,
            tc.tile_pool(name="small", bufs=3) as spool,
            tc.tile_pool(name="psag", bufs=2, space="PSUM") as psag,
            tc.tile_pool(name="psmlp", bufs=2, space="PSUM") as psmlp,
        ):
            colf_s = cpool.tile([P, BLOCKS_PER_CORE * K], f32, tag="colf", name="colf")
            nc.sync.dma_start(out=colf_s[:], in_=colf[:])
            natT_s = cpool.tile([P, NODES_PER_CORE], f32, tag="natT", name="natT")
            nc.sync.dma_start(out=natT_s[:], in_=natT[:])
            iota_s = cpool.tile([P, K, 128], f32, tag="iota", name="iota")
            nc.sync.dma_start(out=iota_s[:], in_=iota[:])
            consts = {}
            for name, t in [("w0a", w0a), ("w0b", w0b), ("w1", w1),
                            ("w2", w2), ("gam", gam), ("bet", bet)]:
                consts[name] = cpool.tile([128, 128], f32, tag=name, name=name)
                nc.sync.dma_start(out=consts[name][:], in_=t[:])
            for name, t in [("b0", b0), ("b1", b1), ("b2", b2)]:
                consts[name] = cpool.tile([128, 1], f32, tag=name, name=name)
                nc.sync.dma_start(out=consts[name][:], in_=t[:])
            ident = cpool.tile([P, P], f32, tag="ident", name="ident")
            make_identity(nc, ident[:])
            epst = cpool.tile([P, 1], f32, tag="eps", name="eps")
            nc.vector.memset(epst[:], EPS)

            loop_cm = (tc.For_i(0, loop_iters, 1) if loop_iters
                       else contextlib.nullcontext())
            with loop_cm:
                _emit_blocks(nc, tc, K, epool, ohpool, spool, psag, psmlp,
                             colf_s, natT_s, iota_s, consts, ident, epst,
                             edges, out, mybir)
    nc.finalize()
    return nc


def _emit_blocks(nc, tc, K, epool, ohpool, spool, psag, psmlp, colf_s,
                 natT_s, iota_s, consts, ident, epst, edges, out, mybir):
    dt = mybir.dt
    f32 = dt.float32
    Alu = mybir.AluOpType
    Act = mybir.ActivationFunctionType
    KE = K * 128
    if True:
        if True:
            for b in range(BLOCKS_PER_CORE):
                eblk = epool.tile([P, KE], f32, tag="eblk", name="eblk")
                nc.sync.dma_start(out=eblk[:], in_=edges[b])

                # one-hot for the whole block in one DVE op:
                # oh[p, k, j] = (colf[p, b*K + k] == j)
                oh = ohpool.tile([P, K, 128], f32, tag="oh", name="oh")
                csl = colf_s[:, b * K:(b + 1) * K].broadcast_to([P, K, 128])
                nc.vector.tensor_tensor(out=oh[:], in0=csl, in1=iota_s[:],
                                        op=Alu.is_equal)

                # aggrT[f, j] = sum_k sum_e edge[e, k*128+f] * oh[e, k, j]
                pag = psag.tile([P, 128], f32, tag="ag", name="ag")
                for k in range(K):
                    nc.tensor.matmul(out=pag[:],
                                     lhsT=eblk[:, k * 128:(k + 1) * 128],
                                     rhs=oh[:, k, :],
                                     start=(k == 0), stop=(k == K - 1))
                aggrT = spool.tile([P, 128], f32, tag="aggrT", name="aggrT")
                nc.scalar.copy(aggrT[:], pag[:])

                # h1T = relu(W0a.T @ natT_blk + W0b.T @ aggrT + b0)
                ph1 = psmlp.tile([P, 128], f32, tag="mlp", name="mlp")
                nc.tensor.matmul(out=ph1[:], lhsT=consts["w0a"][:],
                                 rhs=natT_s[:, b * 128:(b + 1) * 128],
                                 start=True, stop=False)
                nc.tensor.matmul(out=ph1[:], lhsT=consts["w0b"][:],
                                 rhs=aggrT[:], start=False, stop=True)
                h1 = spool.tile([P, 128], f32, tag="h1", name="h1")
                nc.scalar.activation(h1[:], ph1[:], Act.Relu,
                                     bias=consts["b0"][:])

                ph2 = psmlp.tile([P, 128], f32, tag="mlp", name="mlp")
                nc.tensor.matmul(out=ph2[:], lhsT=consts["w1"][:], rhs=h1[:],
                                 start=True, stop=True)
                h2 = spool.tile([P, 128], f32, tag="h2", name="h2")
                nc.scalar.activation(h2[:], ph2[:], Act.Relu,
                                     bias=consts["b1"][:])

                ph3 = psmlp.tile([P, 128], f32, tag="mlp", name="mlp")
                nc.tensor.matmul(out=ph3[:], lhsT=consts["w2"][:], rhs=h2[:],
                                 start=True, stop=True)
                h3T = spool.tile([P, 128], f32, tag="h3T", name="h3T")
                nc.scalar.activation(h3T[:], ph3[:], Act.Identity,
                                     bias=consts["b2"][:])

                # transpose to node-major, then LayerNorm over features
                py = psmlp.tile([P, 128], f32, tag="mlp", name="mlp")
                nc.tensor.transpose(py[:], h3T[:], ident[:])
                y = spool.tile([P, 128], f32, tag="y", name="y")
                nc.scalar.copy(y[:], py[:])

                stats = spool.tile([P, 6], f32, tag="stats", name="stats")
                nc.vector.bn_stats(stats[:], y[:])
                mv = spool.tile([P, 2], f32, tag="mv", name="mv")
                nc.vector.bn_aggr(mv[:], stats[:])
                std = spool.tile([P, 1], f32, tag="std", name="std")
                nc.scalar.activation(std[:], mv[:, 1:2], Act.Sqrt,
                                     bias=epst[:])
                rstd = spool.tile([P, 1], f32, tag="rstd", name="rstd")
                nc.vector.reciprocal(rstd[:], std[:])
                xn = spool.tile([P, 128], f32, tag="xn", name="xn")
                nc.vector.tensor_scalar(out=xn[:], in0=y[:],
                                        scalar1=mv[:, 0:1], scalar2=rstd[:],
                                        op0=Alu.subtract, op1=Alu.mult)
                g1 = spool.tile([P, 128], f32, tag="g1", name="g1")
                nc.gpsimd.tensor_tensor(out=g1[:], in0=xn[:],
                                        in1=consts["gam"][:], op=Alu.mult)
                yo = spool.tile([P, 128], f32, tag="yo", name="yo")
                nc.gpsimd.tensor_tensor(out=yo[:], in0=g1[:],
                                        in1=consts["bet"][:], op=Alu.add)
                nc.sync.dma_start(out=out[b], in_=yo[:])


def _prepare_shards(node_attr, edge_attr, col):
    """Bucket edges by destination-node block; build per-core arrays."""
    E = col.shape[0]
    blk = col >> 7                                  # global block id
    counts = np.bincount(blk, minlength=TOTAL_BLOCKS)
    K = int(np.ceil(max(int(counts.max()), 1) / 128))
    KE = K * 128
    order = np.argsort(blk, kind="stable")
    starts = np.zeros(TOTAL_BLOCKS + 1, np.int64)
    starts[1:] = np.cumsum(counts)
    blk_sorted = blk[order]
    within = np.arange(E, dtype=np.int64) - starts[blk_sorted]
    col_local_sorted = (col[order] & 127).astype(np.float32)

    edges_by_core = []
    colf_by_core = []
    natp = np.zeros((N_CORES * NODES_PER_CORE, D), np.float32)
    natp[:NUM_NODES] = node_attr
    natT_by_core = []
    for c in range(N_CORES):
        lo = int(starts[c * BLOCKS_PER_CORE])
        hi = int(starts[(c + 1) * BLOCKS_PER_CORE])
        loc_blk = blk_sorted[lo:hi] - c * BLOCKS_PER_CORE
        slot = loc_blk * KE + within[lo:hi]
        ebuf = np.zeros((BLOCKS_PER_CORE * KE, D), np.float32)
        ebuf[slot] = edge_attr[order[lo:hi]]
        e4 = ebuf.reshape(BLOCKS_PER_CORE, K, 128, D).transpose(0, 2, 1, 3)
        edges_by_core.append(
            np.ascontiguousarray(e4).reshape(BLOCKS_PER_CORE, P, KE))
        cbuf = np.full((BLOCKS_PER_CORE * KE,), -1.0, np.float32)
        cbuf[slot] = col_local_sorted[lo:hi]
        c4 = cbuf.reshape(BLOCKS_PER_CORE, K, 128).transpose(2, 0, 1)
        colf_by_core.append(
            np.ascontiguousarray(c4).reshape(P, BLOCKS_PER_CORE * K))
        natT_by_core.append(np.ascontiguousarray(
            natp[c * NODES_PER_CORE:(c + 1) * NODES_PER_CORE].T))
    return K, edges_by_core, colf_by_core, natT_by_core


def kernel(node_attr, edge_attr, edge_index, W0, b0, W1, b1, W2, b2,
           ln_g, ln_b):
    from concourse import bass_utils

    node_attr = np.ascontiguousarray(np.asarray(node_attr, dtype=np.float32))
    edge_attr = np.ascontiguousarray(np.asarray(edge_attr, dtype=np.float32))
    col = np.asarray(edge_index)[1].astype(np.int64)
    W0 = np.asarray(W0, dtype=np.float32)
    W1 = np.ascontiguousarray(np.asarray(W1, dtype=np.float32))
    W2 = np.ascontiguousarray(np.asarray(W2, dtype=np.float32))
    b0v = np.asarray(b0, dtype=np.float32).reshape(128, 1).copy()
    b1v = np.asarray(b1, dtype=np.float32).reshape(128, 1).copy()
    b2v = np.asarray(b2, dtype=np.float32).reshape(128, 1).copy()
    gam = np.ascontiguousarray(
        np.tile(np.asarray(ln_g, np.float32).reshape(1, 128), (128, 1)))
    bet = np.ascontiguousarray(
        np.tile(np.asarray(ln_b, np.float32).reshape(1, 128), (128, 1)))

    K, edges_by_core, colf_by_core, natT_by_core = _prepare_shards(
        node_attr, edge_attr, col)

    iota_rep = np.ascontiguousarray(
        np.broadcast_to(np.arange(128, dtype=np.float32), (P, K, 128)))
    w0a = np.ascontiguousarray(W0[:128])
    w0b = np.ascontiguousarray(W0[128:])

    if K not in _nc_cache:
        _nc_cache[K] = _build_nc(K)
    nc = _nc_cache[K]

    shared = {"iota": iota_rep, "w0a": w0a, "w0b": w0b, "w1": W1, "w2": W2,
              "b0": b0v, "b1": b1v, "b2": b2v, "gam": gam, "bet": bet}
    in_maps = []
    for c in range(N_CORES):
        m = {"edges": edges_by_core[c], "colf": colf_by_core[c],
             "natT": natT_by_core[c]}
        m.update(shared)
        in_maps.append(m)

    res = bass_utils.run_bass_kernel_spmd(nc, in_maps,
                                          core_ids=list(range(N_CORES)))
    last_run_info["results"] = res
    last_run_info["nc"] = nc
    last_run_info["in_maps"] = in_maps
    last_run_info["K"] = K

    outs = [res.results[c]["out"].reshape(NODES_PER_CORE, D)
            for c in range(N_CORES)]
    return np.concatenate(outs, axis=0)[:NUM_NODES].astype(np.float32)
